# revision 34
# baseline (speedup 1.0000x reference)
"""Trainium2 Bass kernel for a dense transformer decoder block (B=4, T=2048,
C=1024, 16 heads x 64, DFF=4096), SPMD across 8 NeuronCores.

Sharding: 8 shards of 1024 tokens = (batch, seq-half). Each core receives a
2048-token "window" xkv = [previous 1024 tokens (zeros for first halves) |
own 1024 tokens] so every core runs the identical program: causal attention
of its 1024 queries against window keys [0 .. 1024+q]. Zero-rows produce
K=V=0 => softmax contributions exp(0)=1 with V=0, cancelled exactly by a
host-provided rowsum correction (1024 for first-half cores, 0 otherwise).

v2: activation transposes via DMA-xbar (dma_start_transpose) instead of the
PE; QKV + Wo projections in fp8e4 with DoubleRow perf mode (weights scaled
x64 on host to clear the e4m3 subnormal zone, compensated exactly in the
PSUM copy-out scalings); FFN stays bf16 for accuracy. LN statistics, softmax
normalization and residuals in fp32.
"""

import os
from contextlib import ExitStack

os.environ.setdefault("MYCRO_LOCAL_CACHE", "1")

import numpy as np
import ml_dtypes

import concourse.bacc as bacc
import concourse.bass as bass
import concourse.mybir as mybir
import concourse.tile as tile
from concourse.bass_utils import run_bass_kernel_spmd

BF16 = ml_dtypes.bfloat16
E4 = ml_dtypes.float8_e4m3
P = 128
C = 1024
H = 16
DH = 64
DFF = 4096
NPAIR = 8   # head pairs
NKT = 8     # C / 128 contraction tiles
NS = 4      # C / 256 DoubleRow contraction steps
NW = 16     # window token tiles (2048 tokens)
NT = 8      # own token tiles (1024 tokens)
ND = 32     # DFF / 128 tiles
EPS = 1e-5
WS = 64.0   # fp8 weight upscale

f32 = mybir.dt.float32
bf16 = mybir.dt.bfloat16
fp8 = mybir.dt.float8e4
FT = mybir.ActivationFunctionType
ALU = mybir.AluOpType
DR = mybir.MatmulPerfMode.DoubleRow


def _build(flags):
    """Build the SPMD program. flags: dict of bools for nonzero biases."""
    nc = bacc.Bacc("TRN2", target_bir_lowering=False, debug=False, num_devices=8)

    xkv = nc.dram_tensor("xkv", [2048, C], f32, kind="ExternalInput")
    wq = nc.dram_tensor("wq", [P, 8192], fp8, kind="ExternalInput")
    wk = nc.dram_tensor("wk", [P, 8192], fp8, kind="ExternalInput")
    wv = nc.dram_tensor("wv", [P, 8192], fp8, kind="ExternalInput")
    wo = nc.dram_tensor("wo", [P, 8192], fp8, kind="ExternalInput")
    w1 = nc.dram_tensor("w1", [P, 32768], bf16, kind="ExternalInput")
    w2 = nc.dram_tensor("w2", [P, 32768], bf16, kind="ExternalInput")
    trilq = nc.dram_tensor("trilq", [P, P], bf16, kind="ExternalInput")
    corr = nc.dram_tensor("corr", [P, 1], f32, kind="ExternalInput")
    qbias = nc.dram_tensor("qbias", [P, NPAIR], f32, kind="ExternalInput")
    kbias = nc.dram_tensor("kbias", [P, NPAIR], f32, kind="ExternalInput")
    b1p = nc.dram_tensor("b1p", [P, ND], f32, kind="ExternalInput")
    bo_row = nc.dram_tensor("bo_row", [P, C], f32, kind="ExternalInput")
    b2_row = nc.dram_tensor("b2_row", [P, C], f32, kind="ExternalInput")
    out = nc.dram_tensor("out", [1024, C], f32, kind="ExternalOutput")

    with tile.TileContext(nc) as tc, ExitStack() as es:
        consts = es.enter_context(tc.tile_pool(name="consts", bufs=1))
        tril_sb = consts.tile([P, P], bf16, tag="tril")
        nc.sync.dma_start(out=tril_sb[:, :], in_=trilq.ap()[:, :])
        corr_sb = consts.tile([P, 1], f32, tag="corr")
        nc.sync.dma_start(out=corr_sb[:, :], in_=corr.ap()[:, :])
        qb_sb = consts.tile([P, NPAIR], f32, tag="qb")
        nc.sync.dma_start(out=qb_sb[:, :], in_=qbias.ap()[:, :])
        kb_sb = consts.tile([P, NPAIR], f32, tag="kb")
        nc.sync.dma_start(out=kb_sb[:, :], in_=kbias.ap()[:, :])
        b1_sb = consts.tile([P, ND], f32, tag="b1")
        nc.sync.dma_start(out=b1_sb[:, :], in_=b1p.ap()[:, :])
        eps_sb = consts.tile([P, 1], f32, tag="eps")
        nc.vector.memset(eps_sb[:, :], EPS)
        if flags["bo"]:
            bo_sb = consts.tile([P, C], f32, tag="bo")
            nc.sync.dma_start(out=bo_sb[:, :], in_=bo_row.ap()[:, :])
        if flags["b2"]:
            b2_sb = consts.tile([P, C], f32, tag="b2")
            nc.sync.dma_start(out=b2_sb[:, :], in_=b2_row.ap()[:, :])

        # persistent activation storage
        qt_pool = es.enter_context(tc.tile_pool(name="qt", bufs=NPAIR))
        kt_pool = es.enter_context(tc.tile_pool(name="kt", bufs=NPAIR))
        v_pool = es.enter_context(tc.tile_pool(name="vv", bufs=NW))
        QT = [qt_pool.tile([P, 1024], bf16, tag="qt", name=f"qt{i}") for i in range(NPAIR)]
        KT = [kt_pool.tile([P, 2048], bf16, tag="kt", name=f"kt{i}") for i in range(NPAIR)]
        # V with interleaved ones columns: per pair 65+65 cols
        VO = [v_pool.tile([P, NPAIR * 130], bf16, tag="vv", name=f"vo{i}") for i in range(NW)]
        x_pool = es.enter_context(tc.tile_pool(name="xx", bufs=NT))
        X = [x_pool.tile([P, C], f32, tag="xx", name=f"xt{i}") for i in range(NT)]

        def ln_tile(src_ap, lnp, zpool):
            """LayerNorm a [128, C] fp32 tile -> bf16 z tile (g/b folded out)."""
            if isinstance(src_ap, tuple):  # (dram_ap,) to load
                xw = lnp.tile([P, C], f32, tag="xw")
                nc.sync.dma_start(out=xw[:, :], in_=src_ap[0])
            else:
                xw = src_ap
            stats = lnp.tile([P, 2, 6], f32, tag="stats")
            nc.vector.bn_stats(out=stats[:, 0, :], in_=xw[:, 0:512])
            nc.vector.bn_stats(out=stats[:, 1, :], in_=xw[:, 512:1024])
            mv = lnp.tile([P, 2], f32, tag="mv")
            nc.vector.bn_aggr(out=mv[:, :], in_=stats[:, :, :])
            rsig = lnp.tile([P, 1], f32, tag="rsig")
            nc.scalar.activation(rsig[:, :], mv[:, 1:2], FT.Sqrt,
                                 bias=eps_sb[:, :], scale=1.0)
            nc.vector.reciprocal(rsig[:, :], rsig[:, :])
            z = zpool.tile([P, C], bf16, tag="z")
            nc.vector.tensor_scalar(z[:, :], xw[:, :], mv[:, 0:1], rsig[:, :],
                                    ALU.subtract, ALU.mult)
            return z

        # ---------------- Phase 1+2: LN1, hT (xbar), QKV fp8 projections ------
        ht8_es = ExitStack()
        ht8_pool = ht8_es.enter_context(tc.tile_pool(name="ht8", bufs=4))
        HT8 = [ht8_pool.tile([P, NKT, 512], fp8, tag="ht8", name=f"ht8_{g}")
               for g in range(4)]
        with tc.tile_pool(name="ln1", bufs=3) as lnp, \
             tc.tile_pool(name="z1", bufs=3) as zpool, \
             tc.tile_pool(name="htt", bufs=3) as htt_pool, \
             tc.tile_pool(name="wqkv", bufs=2) as wqkv_pool, \
             tc.tile_pool(name="wvp", bufs=NS) as wv_pool, \
             tc.tile_pool(name="qkvps", bufs=4, space="PSUM") as qkvps, \
             tc.tile_pool(name="vps", bufs=2, space="PSUM") as vps:

            WV8 = [wv_pool.tile([P, 2, 1024], fp8, tag="wv", name=f"wvt{i}") for i in range(NS)]
            for s in range(NS):
                nc.sync.dma_start(out=WV8[s][:, :, :],
                                  in_=wv.ap()[:, s * 2048:(s + 1) * 2048])
            for g in range(4):
                for w in range(4 * g, 4 * g + 4):
                    z = ln_tile((xkv.ap()[w * P:(w + 1) * P, :],), lnp, zpool)
                    htt = htt_pool.tile([P, NKT, P], bf16, tag="htt")
                    nc.scalar.dma_start_transpose(htt[:, :, :], z[:, :])
                    nc.scalar.copy(out=HT8[g][:, :, (w % 4) * P:(w % 4 + 1) * P],
                                   in_=htt[:, :, :])
                # V projection for this group (fp8 DoubleRow, lhsT reused)
                for w in range(4 * g, 4 * g + 4):
                    wc = (w % 4) * P
                    pv = vps.tile([P, 1024], f32, tag="vps")
                    for s in range(NS):
                        st, sp = s == 0, s == NS - 1
                        nc.tensor.ldweights(
                            HT8[g][:, 2 * s:2 * s + 2, wc:wc + P],
                            perf_mode=DR)
                        for hf in range(2):
                            mm = nc.tensor.matmul(
                                pv[:, hf * 512:(hf + 1) * 512],
                                HT8[g][:, 2 * s:2 * s + 2, wc:wc + P],
                                WV8[s][:, :, hf * 512:(hf + 1) * 512],
                                start=st, stop=sp, perf_mode=DR)
                            mm.ldweights = False
                    # scatter into [pair, hi, 64] slots (ones cols untouched)
                    # VO holds 64*V (weights upscaled); fixed in softmax recip
                    vdst = VO[w][:, :].rearrange("p (pr hi dd) -> p pr hi dd",
                                                 pr=NPAIR, hi=2)[:, :, :, 0:64]
                    vsrc = pv[:, :].rearrange("p (pr hi dd) -> p pr hi dd",
                                              pr=NPAIR, hi=2)
                    nc.vector.tensor_copy(out=vdst, in_=vsrc)
                    ones = VO[w][:, :].rearrange("p (pr hi dd) -> p pr hi dd",
                                                 pr=NPAIR, hi=2)[:, :, :, 64:65]
                    nc.vector.memset(ones, 1.0)

            # K then Q per pair; s-outer loops with parallel PSUM accum groups
            for pr in range(NPAIR):
                wk_sb = wqkv_pool.tile([P, NS, 2, P], fp8, tag="wqk")
                nc.sync.dma_start(out=wk_sb[:, :, :, :],
                                  in_=wk.ap()[:, pr * 1024:(pr + 1) * 1024])
                pks = [qkvps.tile([P, 512], f32, tag="qkvps", name=f"pk{i}")
                       for i in range(4)]
                for s in range(NS):
                    nc.tensor.ldweights(wk_sb[:, s, :, :], perf_mode=DR)
                    for wh in range(4):
                        mm = nc.tensor.matmul(
                            pks[wh][:, :], wk_sb[:, s, :, :],
                            HT8[wh][:, 2 * s:2 * s + 2, :],
                            start=(s == 0), stop=(s == NS - 1), perf_mode=DR)
                        mm.ldweights = False
                for wh in range(4):
                    # KT = (pk + 64*kb) / 64
                    nc.vector.tensor_scalar(
                        KT[pr][:, wh * 512:(wh + 1) * 512], pks[wh][:, :],
                        kb_sb[:, pr:pr + 1], 1.0 / WS, ALU.add, ALU.mult)
                wq_sb = wqkv_pool.tile([P, NS, 2, P], fp8, tag="wqk")
                nc.sync.dma_start(out=wq_sb[:, :, :, :],
                                  in_=wq.ap()[:, pr * 1024:(pr + 1) * 1024])
                pqs = [vps.tile([P, 512], f32, tag="vps", name=f"pq{i}")
                       for i in range(2)]
                for s in range(NS):
                    nc.tensor.ldweights(wq_sb[:, s, :, :], perf_mode=DR)
                    for qh in range(2):
                        mm = nc.tensor.matmul(
                            pqs[qh][:, :], wq_sb[:, s, :, :],
                            HT8[2 + qh][:, 2 * s:2 * s + 2, :],
                            start=(s == 0), stop=(s == NS - 1), perf_mode=DR)
                        mm.ldweights = False
                for qh in range(2):
                    # QT = (pq + 64*qb) * (0.125/64)
                    nc.vector.tensor_scalar(
                        QT[pr][:, qh * 512:(qh + 1) * 512], pqs[qh][:, :],
                        qb_sb[:, pr:pr + 1], 0.125 / WS, ALU.add, ALU.mult)
        ht8_es.close()

        for it in range(NT):
            nc.sync.dma_start(out=X[it][:, :],
                              in_=xkv.ap()[1024 + it * P:1024 + (it + 1) * P, :])
        # ---------------- Phase 3: attention ----------------
        o_es = ExitStack()
        o_pool = o_es.enter_context(tc.tile_pool(name="oo", bufs=NT, side="right"))
        O = [o_pool.tile([P, C], bf16, tag="oo", name=f"ot{i}") for i in range(NT)]
        with tc.tile_pool(name="ep", bufs=4) as ep_pool, \
             tc.tile_pool(name="sal", bufs=4) as sal_pool, \
             tc.tile_pool(name="sps", bufs=2, space="PSUM") as sps, \
             tc.tile_pool(name="ops", bufs=4, space="PSUM") as ops:
            for pr in range(NPAIR):
                for ih in range(2):
                    q_lo, q_hi = ih * 512, (ih + 1) * 512
                    # two q-tiles share one opsum bank (single accum group
                    # per bank: one start, one stop)
                    opsum, opcol = {}, {}
                    for g in range(2):
                        t = ops.tile([P, 260], f32, tag="ops", name=f"ops{g}")
                        for k in range(2):
                            it = ih * 4 + g * 2 + k
                            opsum[it] = t
                            opcol[it] = k * 130
                    jmax = ih * 4 + 3 + 8
                    for j in range(jmax + 1):
                        q0 = max((j - 8) * P, q_lo)
                        qlen = q_hi - q0
                        sp = sps.tile([P, 512 + qlen], f32, tag="sps")
                        for hi in range(2):
                            lh = KT[pr][hi * 64:(hi + 1) * 64,
                                         j * P:(j + 1) * P]
                            nc.tensor.matmul(
                                sp[:, hi * 512:hi * 512 + qlen],
                                lh, QT[pr][hi * 64:(hi + 1) * 64, q0:q_hi],
                                start=True, stop=True)
                        ep = ep_pool.tile([P, 2 * qlen], bf16, tag="ep")
                        if qlen == 512:
                            nc.scalar.activation(ep[:, :], sp[:, :], FT.Exp)
                        else:
                            for hi in range(2):
                                nc.scalar.activation(
                                    ep[:, hi * qlen:(hi + 1) * qlen],
                                    sp[:, hi * 512:hi * 512 + qlen], FT.Exp)
                        if j - 8 >= ih * 4 and j >= 8:
                            # diagonal block: mask with transposed-tril
                            for hi in range(2):
                                sl = ep[:, hi * qlen:hi * qlen + P]
                                nc.vector.tensor_mul(sl, sl, tril_sb[:, :])
                        for it in range(max(ih * 4, j - 8), ih * 4 + 4):
                            off = it * P - q0
                            for hi in range(2):
                                # bank group: first writer (even it) starts,
                                # last writer (odd it at its last j) stops
                                nc.tensor.matmul(
                                    opsum[it][:, opcol[it] + hi * 65:
                                              opcol[it] + (hi + 1) * 65],
                                    ep[:, hi * qlen + off:hi * qlen + off + P],
                                    VO[j][:, pr * 130 + hi * 65:
                                          pr * 130 + (hi + 1) * 65],
                                    start=(j == 0 and hi == 0 and (it & 1) == 0),
                                    stop=(j == it + 8 and hi == 1 and
                                          (it & 1) == 1))
                    for it in range(ih * 4, ih * 4 + 4):
                        for hi in range(2):
                            rs = sal_pool.tile([P, 1], f32, tag="rs")
                            # V cols hold 64*V: rs = 1/(64*(rowsum - corr))
                            nc.vector.tensor_scalar(
                                rs[:, :],
                                opsum[it][:, opcol[it] + hi * 65 + 64:
                                          opcol[it] + hi * 65 + 65],
                                corr_sb[:, :], WS, ALU.subtract, ALU.mult)
                            nc.vector.reciprocal(rs[:, :], rs[:, :])
                            nc.vector.tensor_scalar_mul(
                                O[it][:, pr * P + hi * 64:pr * P + hi * 64 + 64],
                                opsum[it][:, opcol[it] + hi * 65:
                                          opcol[it] + hi * 65 + 64], rs[:, :])

        # ---------------- Phase 4: O^T (xbar), Wo fp8, residual ----------------
        with tc.tile_pool(name="ott", bufs=3) as ott_pool, \
             tc.tile_pool(name="ot8", bufs=1) as ot8_pool, \
             tc.tile_pool(name="wos", bufs=1) as wo_pool, \
             tc.tile_pool(name="wops", bufs=2, space="PSUM") as wops:
            OT8 = ot8_pool.tile([P, NKT, 1024], fp8, tag="ot8")
            wo_sb = wo_pool.tile([P, NS, 2, 1024], fp8, tag="wo")
            nc.sync.dma_start(out=wo_sb[:, :, :, :], in_=wo.ap()[:, :])
            for it in range(NT):
                ott = ott_pool.tile([P, NKT, P], bf16, tag="ott")
                nc.scalar.dma_start_transpose(ott[:, :, :], O[it][:, :])
                nc.scalar.copy(out=OT8[:, :, it * P:(it + 1) * P],
                               in_=ott[:, :, :])
            for it in range(NT):
                pw = wops.tile([P, 1024], f32, tag="wops")
                for s in range(NS):
                    nc.tensor.ldweights(
                        OT8[:, 2 * s:2 * s + 2, it * P:(it + 1) * P],
                        perf_mode=DR)
                    for hf in range(2):
                        mm = nc.tensor.matmul(
                            pw[:, hf * 512:(hf + 1) * 512],
                            OT8[:, 2 * s:2 * s + 2, it * P:(it + 1) * P],
                            wo_sb[:, s, :, hf * 512:(hf + 1) * 512],
                            start=(s == 0), stop=(s == NS - 1), perf_mode=DR)
                        mm.ldweights = False
                # X += pw/64 (wo upscaled)
                nc.vector.scalar_tensor_tensor(
                    out=X[it][:, :], in0=pw[:, :], scalar=1.0 / WS,
                    in1=X[it][:, :], op0=ALU.mult, op1=ALU.add)
                if flags["bo"]:
                    nc.vector.tensor_add(X[it][:, :], X[it][:, :], bo_sb[:, :])
        o_es.close()

        # ---------------- Phase 5+6: LN2, FFN (bf16), residual, store ----------
        with tc.tile_pool(name="ln2", bufs=3) as lnp2, \
             tc.tile_pool(name="z2", bufs=3) as zpool2, \
             tc.tile_pool(name="h2t", bufs=2) as h2t_pool, \
             tc.tile_pool(name="ut", bufs=ND) as ut_pool, \
             tc.tile_pool(name="w1s", bufs=8) as w1_pool, \
             tc.tile_pool(name="w2s", bufs=8) as w2_pool, \
             tc.tile_pool(name="ups", bufs=2, space="PSUM") as ups, \
             tc.tile_pool(name="yps", bufs=4, space="PSUM") as yps:
            H2T = [h2t_pool.tile([P, NKT, 512], bf16, tag="h2t",
                                 name=f"h2t{h}") for h in range(2)]
            for it in range(4):
                z2 = ln_tile(X[it], lnp2, zpool2)
                nc.scalar.dma_start_transpose(
                    H2T[0][:, :, it * P:(it + 1) * P], z2[:, :])
            for tch in range(2):
                UT = [ut_pool.tile([P, 512], bf16, tag="ut", name=f"ut{i}") for i in range(ND)]
                for d in range(ND):
                    w1_sb = w1_pool.tile([P, 1024], bf16, tag="w1")
                    nc.sync.dma_start(
                        out=w1_sb[:, :],
                        in_=w1.ap()[:, d * 1024:(d + 1) * 1024])
                    pu = ups.tile([P, 512], f32, tag="ups")
                    for kt in range(NKT):
                        nc.tensor.matmul(
                            pu[:, :], w1_sb[:, kt * P:(kt + 1) * P],
                            H2T[tch][:, kt, :],
                            start=(kt == 0), stop=(kt == NKT - 1))
                    # relu(x + b1) on the ACT engine
                    nc.scalar.activation(UT[d][:, :], pu[:, :], FT.Relu,
                                         bias=b1_sb[:, d:d + 1], scale=1.0)
                if tch == 0:
                    # LN2 for the second half runs on DVE under W1/W2 PE work
                    for it in range(4, 8):
                        z2 = ln_tile(X[it], lnp2, zpool2)
                        nc.scalar.dma_start_transpose(
                            H2T[1][:, :, (it - 4) * P:(it - 3) * P], z2[:, :])
                for ch in range(2):
                    ypsum = [yps.tile([P, 512], f32, tag="yps", name=f"yps{i}")
                             for i in range(4)]
                    for d in range(ND):
                        w2_sb = w2_pool.tile([P, 512], bf16, tag="w2")
                        nc.sync.dma_start(
                            out=w2_sb[:, :],
                            in_=w2.ap()[:, d * 1024 + ch * 512:
                                        d * 1024 + (ch + 1) * 512])
                        for tt in range(4):
                            nc.tensor.matmul(
                                ypsum[tt][:, :],
                                UT[d][:, tt * P:(tt + 1) * P],
                                w2_sb[:, :],
                                start=(d == 0), stop=(d == ND - 1))
                    for tt in range(4):
                        it = tch * 4 + tt
                        xsl = X[it][:, ch * 512:(ch + 1) * 512]
                        nc.vector.tensor_add(xsl, ypsum[tt][:, :], xsl)
                        if flags["b2"]:
                            nc.vector.tensor_add(
                                xsl, xsl, b2_sb[:, ch * 512:(ch + 1) * 512])
                for tt in range(4):
                    it = tch * 4 + tt
                    nc.sync.dma_start(out=out.ap()[it * P:(it + 1) * P, :],
                                      in_=X[it][:, :])

    nc.compile()
    return nc


_CACHE = {}


def _prep(inputs):
    """Host-side preprocessing: fold LN affine into weights, tile/cast, shard."""
    x = np.asarray(inputs["x"], np.float32)
    Wq = np.asarray(inputs["Wq"], np.float32)
    Wk = np.asarray(inputs["Wk"], np.float32)
    Wv = np.asarray(inputs["Wv"], np.float32)
    Wo = np.asarray(inputs["Wo"], np.float32)
    bo = np.asarray(inputs["bo"], np.float32)
    W1 = np.asarray(inputs["W1"], np.float32)
    b1 = np.asarray(inputs["b1"], np.float32)
    W2 = np.asarray(inputs["W2"], np.float32)
    b2 = np.asarray(inputs["b2"], np.float32)
    g1 = np.asarray(inputs["g1"], np.float32)
    be1 = np.asarray(inputs["be1"], np.float32)
    g2 = np.asarray(inputs["g2"], np.float32)
    be2 = np.asarray(inputs["be2"], np.float32)

    Wq_g = (Wq * g1[None, :, None] * WS).astype(E4)   # [16,1024,64], x64
    Wk_g = (Wk * g1[None, :, None] * WS).astype(E4)
    Wv_g = (Wv * g1[None, :, None] * WS).astype(E4)
    Wo_s = (Wo * WS).astype(E4)
    # bias folds use the quantized weights so the compensation is exact
    qb = np.einsum('c,hcd->hd', be1, Wq_g.astype(np.float32))  # 64*qb [16,64]
    kb = np.einsum('c,hcd->hd', be1, Wk_g.astype(np.float32))
    vb = np.einsum('c,hcd->hd', be1, Wv_g.astype(np.float32))
    if np.abs(vb).max() > 0:
        raise NotImplementedError("nonzero folded V bias not supported")

    def lhsT_pack_dr(wflat):
        # [1024 c, 1024 m] -> [128 c_lo, (pair, s, two, 128 m)]
        w = wflat.reshape(NS, 2, P, NPAIR, P)          # [s, two, c_lo, pair, m]
        return np.ascontiguousarray(
            w.transpose(2, 3, 0, 1, 4).reshape(P, 8192))

    def rhs_pack_dr(wflat):
        # [1024 k, 1024 n] -> [128 k_lo, (s, two, 1024 n)]
        w = wflat.reshape(NS, 2, P, 1024)              # [s, two, k_lo, n]
        return np.ascontiguousarray(
            w.transpose(2, 0, 1, 3).reshape(P, 8192))

    wq_h = lhsT_pack_dr(Wq_g.transpose(1, 0, 2).reshape(1024, 1024))
    wk_h = lhsT_pack_dr(Wk_g.transpose(1, 0, 2).reshape(1024, 1024))
    wv_h = rhs_pack_dr(Wv_g.transpose(1, 0, 2).reshape(1024, 1024))
    wo_h = rhs_pack_dr(Wo_s)
    W1_g = (W1 * g2[:, None]).astype(BF16)         # [1024, 4096]
    b1p = b1 + be2 @ W1_g.astype(np.float32)
    w1_h = np.ascontiguousarray(
        W1_g.reshape(8, 128, 32, 128).transpose(1, 2, 0, 3).reshape(128, 32768))
    w2_h = np.ascontiguousarray(
        W2.astype(BF16).reshape(32, 128, 1024).transpose(1, 0, 2).reshape(128, 32768))

    # per-pair stacked [128, 8] bias tables (pre-scaled by 64 via Wq_g/Wk_g)
    qb_t = np.zeros((128, 8), np.float32)
    kb_t = np.zeros((128, 8), np.float32)
    for pr in range(8):
        qb_t[0:64, pr] = qb[2 * pr]
        qb_t[64:128, pr] = qb[2 * pr + 1]
        kb_t[0:64, pr] = kb[2 * pr]
        kb_t[64:128, pr] = kb[2 * pr + 1]
    b1_t = np.ascontiguousarray(b1p.reshape(32, 128).T.astype(np.float32))
    bo_t = np.broadcast_to(bo, (128, 1024)).astype(np.float32).copy()
    b2_t = np.broadcast_to(b2, (128, 1024)).astype(np.float32).copy()

    tril = np.triu(np.ones((128, 128), np.float32)).astype(BF16)

    flags = {"bo": bool(np.abs(bo).max() > 0), "b2": bool(np.abs(b2).max() > 0)}

    shared = dict(wq=wq_h, wk=wk_h, wv=wv_h, wo=wo_h, w1=w1_h, w2=w2_h,
                  trilq=tril, qbias=qb_t, kbias=kb_t,
                  b1p=b1_t, bo_row=bo_t, b2_row=b2_t)
    in_maps = []
    for core in range(8):
        b, half = core // 2, core % 2
        xw = np.zeros((2048, 1024), np.float32)
        if half == 1:
            xw[:1024] = x[b, :1024]
        xw[1024:] = x[b, half * 1024:(half + 1) * 1024]
        cr = np.full((128, 1), 1024.0 if half == 0 else 0.0, np.float32)
        in_maps.append({"xkv": xw, "corr": cr, **shared})
    return in_maps, flags


def _get_nc(flags):
    key = tuple(sorted(flags.items()))
    if key not in _CACHE:
        _CACHE[key] = _build(flags)
    return _CACHE[key]


def run(inputs, **kw):
    in_maps, flags = _prep(inputs)
    nc = _get_nc(flags)
    res = run_bass_kernel_spmd(nc, in_maps, core_ids=list(range(8)), **kw)
    x = np.asarray(inputs["x"], np.float32)
    outf = np.zeros_like(x)
    for core in range(8):
        b, half = core // 2, core % 2
        outf[b, half * 1024:(half + 1) * 1024] = res.results[core]["out"]
    return outf, res


def kernel(**inputs):
    outf, _ = run(inputs)
    return outf


# revision 35
# speedup vs baseline: 1.1614x; 1.1614x over previous
"""Trainium2 Bass kernel for a dense transformer decoder block (B=4, T=2048,
C=1024, 16 heads x 64, DFF=4096), SPMD across 8 NeuronCores.

Sharding: 8 shards of 1024 tokens = (batch, seq-half). Each core receives a
2048-token "window" xkv = [previous 1024 tokens (zeros for first halves) |
own 1024 tokens] so every core runs the identical program: causal attention
of its 1024 queries against window keys [0 .. 1024+q]. Zero-rows produce
K=V=0 => softmax contributions exp(0)=1 with V=0, cancelled exactly by a
host-provided rowsum correction (1024 for first-half cores, 0 otherwise).

v2: activation transposes via DMA-xbar (dma_start_transpose) instead of the
PE; QKV + Wo projections in fp8e4 with DoubleRow perf mode (weights scaled
x64 on host to clear the e4m3 subnormal zone, compensated exactly in the
PSUM copy-out scalings); FFN stays bf16 for accuracy. LN statistics, softmax
normalization and residuals in fp32.
"""

import os
from contextlib import ExitStack

os.environ.setdefault("MYCRO_LOCAL_CACHE", "1")

import numpy as np
import ml_dtypes

import concourse.bacc as bacc
import concourse.bass as bass
import concourse.mybir as mybir
import concourse.tile as tile
from concourse.bass_utils import run_bass_kernel_spmd

BF16 = ml_dtypes.bfloat16
E4 = ml_dtypes.float8_e4m3
P = 128
C = 1024
H = 16
DH = 64
DFF = 4096
NPAIR = 8   # head pairs
NKT = 8     # C / 128 contraction tiles
NS = 4      # C / 256 DoubleRow contraction steps
NW = 16     # window token tiles (2048 tokens)
NT = 8      # own token tiles (1024 tokens)
ND = 32     # DFF / 128 tiles
EPS = 1e-5
WS = 64.0   # fp8 weight upscale

f32 = mybir.dt.float32
bf16 = mybir.dt.bfloat16
fp8 = mybir.dt.float8e4
FT = mybir.ActivationFunctionType
ALU = mybir.AluOpType
DR = mybir.MatmulPerfMode.DoubleRow


def _build(flags):
    """Build the SPMD program. flags: dict of bools for nonzero biases."""
    nc = bacc.Bacc("TRN2", target_bir_lowering=False, debug=False, num_devices=8)

    xkv = nc.dram_tensor("xkv", [2048, C], f32, kind="ExternalInput")
    wq = nc.dram_tensor("wq", [P, 8192], fp8, kind="ExternalInput")
    wk = nc.dram_tensor("wk", [P, 8192], fp8, kind="ExternalInput")
    wv = nc.dram_tensor("wv", [P, 8192], fp8, kind="ExternalInput")
    wo = nc.dram_tensor("wo", [P, 8192], fp8, kind="ExternalInput")
    w1 = nc.dram_tensor("w1", [P, 32768], bf16, kind="ExternalInput")
    w2 = nc.dram_tensor("w2", [P, 32768], bf16, kind="ExternalInput")
    trilq = nc.dram_tensor("trilq", [P, P], bf16, kind="ExternalInput")
    corr = nc.dram_tensor("corr", [P, 1], f32, kind="ExternalInput")
    qbias = nc.dram_tensor("qbias", [P, NPAIR], f32, kind="ExternalInput")
    kbias = nc.dram_tensor("kbias", [P, NPAIR], f32, kind="ExternalInput")
    b1p = nc.dram_tensor("b1p", [P, ND], f32, kind="ExternalInput")
    bo_row = nc.dram_tensor("bo_row", [P, C], f32, kind="ExternalInput")
    b2_row = nc.dram_tensor("b2_row", [P, C], f32, kind="ExternalInput")
    out = nc.dram_tensor("out", [1024, C], f32, kind="ExternalOutput")

    with tile.TileContext(nc) as tc, ExitStack() as es:
        consts = es.enter_context(tc.tile_pool(name="consts", bufs=1))
        tril_sb = consts.tile([P, P], bf16, tag="tril")
        nc.sync.dma_start(out=tril_sb[:, :], in_=trilq.ap()[:, :])
        corr_sb = consts.tile([P, 1], f32, tag="corr")
        nc.sync.dma_start(out=corr_sb[:, :], in_=corr.ap()[:, :])
        qb_sb = consts.tile([P, NPAIR], f32, tag="qb")
        nc.sync.dma_start(out=qb_sb[:, :], in_=qbias.ap()[:, :])
        kb_sb = consts.tile([P, NPAIR], f32, tag="kb")
        nc.sync.dma_start(out=kb_sb[:, :], in_=kbias.ap()[:, :])
        b1_sb = consts.tile([P, ND], f32, tag="b1")
        nc.sync.dma_start(out=b1_sb[:, :], in_=b1p.ap()[:, :])
        eps_sb = consts.tile([P, 1], f32, tag="eps")
        nc.vector.memset(eps_sb[:, :], EPS)
        if flags["bo"]:
            bo_sb = consts.tile([P, C], f32, tag="bo")
            nc.sync.dma_start(out=bo_sb[:, :], in_=bo_row.ap()[:, :])
        if flags["b2"]:
            b2_sb = consts.tile([P, C], f32, tag="b2")
            nc.sync.dma_start(out=b2_sb[:, :], in_=b2_row.ap()[:, :])

        # persistent activation storage
        qt_pool = es.enter_context(tc.tile_pool(name="qt", bufs=NPAIR))
        kt_pool = es.enter_context(tc.tile_pool(name="kt", bufs=NPAIR))
        v_pool = es.enter_context(tc.tile_pool(name="vv", bufs=NW))
        QT = [qt_pool.tile([P, 1024], bf16, tag="qt", name=f"qt{i}") for i in range(NPAIR)]
        KT = [kt_pool.tile([P, 2048], bf16, tag="kt", name=f"kt{i}") for i in range(NPAIR)]
        # V with interleaved ones columns: per pair 65+65 cols
        VO = [v_pool.tile([P, NPAIR * 130], bf16, tag="vv", name=f"vo{i}") for i in range(NW)]
        x_pool = es.enter_context(tc.tile_pool(name="xx", bufs=NT))
        X = [x_pool.tile([P, C], f32, tag="xx", name=f"xt{i}") for i in range(NT)]

        def ln_tile(src_ap, lnp, zpool):
            """LayerNorm a [128, C] fp32 tile -> bf16 z tile (g/b folded out)."""
            if isinstance(src_ap, tuple):  # (dram_ap,) to load
                xw = lnp.tile([P, C], f32, tag="xw")
                nc.sync.dma_start(out=xw[:, :], in_=src_ap[0])
            else:
                xw = src_ap
            stats = lnp.tile([P, 2, 6], f32, tag="stats")
            nc.vector.bn_stats(out=stats[:, 0, :], in_=xw[:, 0:512])
            nc.vector.bn_stats(out=stats[:, 1, :], in_=xw[:, 512:1024])
            mv = lnp.tile([P, 2], f32, tag="mv")
            nc.vector.bn_aggr(out=mv[:, :], in_=stats[:, :, :])
            rsig = lnp.tile([P, 1], f32, tag="rsig")
            nc.scalar.activation(rsig[:, :], mv[:, 1:2], FT.Sqrt,
                                 bias=eps_sb[:, :], scale=1.0)
            nc.vector.reciprocal(rsig[:, :], rsig[:, :])
            z = zpool.tile([P, C], bf16, tag="z")
            nc.vector.tensor_scalar(z[:, :], xw[:, :], mv[:, 0:1], rsig[:, :],
                                    ALU.subtract, ALU.mult)
            return z

        # ---------------- Phase 1+2: LN1, hT (xbar), QKV fp8 projections ------
        ht8_es = ExitStack()
        ht8_pool = ht8_es.enter_context(tc.tile_pool(name="ht8", bufs=4))
        HT8 = [ht8_pool.tile([P, NKT, 512], fp8, tag="ht8", name=f"ht8_{g}")
               for g in range(4)]
        with tc.tile_pool(name="ln1", bufs=3) as lnp, \
             tc.tile_pool(name="z1", bufs=3) as zpool, \
             tc.tile_pool(name="htt", bufs=3) as htt_pool, \
             tc.tile_pool(name="wqkv", bufs=2) as wqkv_pool, \
             tc.tile_pool(name="wvp", bufs=NS) as wv_pool, \
             tc.tile_pool(name="qkvps", bufs=4, space="PSUM") as qkvps, \
             tc.tile_pool(name="vps", bufs=2, space="PSUM") as vps:

            WV8 = [wv_pool.tile([P, 2, 1024], fp8, tag="wv", name=f"wvt{i}") for i in range(NS)]
            for s in range(NS):
                nc.sync.dma_start(out=WV8[s][:, :, :],
                                  in_=wv.ap()[:, s * 2048:(s + 1) * 2048])
            for g in range(4):
                for w in range(4 * g, 4 * g + 4):
                    z = ln_tile((xkv.ap()[w * P:(w + 1) * P, :],), lnp, zpool)
                    htt = htt_pool.tile([P, NKT, P], bf16, tag="htt")
                    nc.scalar.dma_start_transpose(htt[:, :, :], z[:, :])
                    nc.scalar.copy(out=HT8[g][:, :, (w % 4) * P:(w % 4 + 1) * P],
                                   in_=htt[:, :, :])
                # V projection for this group (fp8 DoubleRow, lhsT reused)
                for w in range(4 * g, 4 * g + 4):
                    wc = (w % 4) * P
                    pv = vps.tile([P, 1024], f32, tag="vps")
                    for s in range(NS):
                        st, sp = s == 0, s == NS - 1
                        nc.tensor.ldweights(
                            HT8[g][:, 2 * s:2 * s + 2, wc:wc + P],
                            perf_mode=DR)
                        for hf in range(2):
                            mm = nc.tensor.matmul(
                                pv[:, hf * 512:(hf + 1) * 512],
                                HT8[g][:, 2 * s:2 * s + 2, wc:wc + P],
                                WV8[s][:, :, hf * 512:(hf + 1) * 512],
                                start=st, stop=sp, perf_mode=DR)
                            mm.ldweights = False
                    # scatter into [pair, hi, 64] slots (ones cols untouched)
                    # VO holds 64*V (weights upscaled); fixed in softmax recip
                    vdst = VO[w][:, :].rearrange("p (pr hi dd) -> p pr hi dd",
                                                 pr=NPAIR, hi=2)[:, :, :, 0:64]
                    vsrc = pv[:, :].rearrange("p (pr hi dd) -> p pr hi dd",
                                              pr=NPAIR, hi=2)
                    nc.vector.tensor_copy(out=vdst, in_=vsrc)
                    ones = VO[w][:, :].rearrange("p (pr hi dd) -> p pr hi dd",
                                                 pr=NPAIR, hi=2)[:, :, :, 64:65]
                    nc.vector.memset(ones, 1.0)

            # K then Q per pair; s-outer loops with parallel PSUM accum groups
            for pr in range(NPAIR):
                wk_sb = wqkv_pool.tile([P, NS, 2, P], fp8, tag="wqk")
                nc.sync.dma_start(out=wk_sb[:, :, :, :],
                                  in_=wk.ap()[:, pr * 1024:(pr + 1) * 1024])
                pks = [qkvps.tile([P, 512], f32, tag="qkvps", name=f"pk{i}")
                       for i in range(4)]
                for s in range(NS):
                    nc.tensor.ldweights(wk_sb[:, s, :, :], perf_mode=DR)
                    for wh in range(4):
                        mm = nc.tensor.matmul(
                            pks[wh][:, :], wk_sb[:, s, :, :],
                            HT8[wh][:, 2 * s:2 * s + 2, :],
                            start=(s == 0), stop=(s == NS - 1), perf_mode=DR)
                        mm.ldweights = False
                for wh in range(4):
                    # KT = (pk + 64*kb) / 64
                    nc.vector.tensor_scalar(
                        KT[pr][:, wh * 512:(wh + 1) * 512], pks[wh][:, :],
                        kb_sb[:, pr:pr + 1], 1.0 / WS, ALU.add, ALU.mult)
                wq_sb = wqkv_pool.tile([P, NS, 2, P], fp8, tag="wqk")
                nc.sync.dma_start(out=wq_sb[:, :, :, :],
                                  in_=wq.ap()[:, pr * 1024:(pr + 1) * 1024])
                pqs = [qkvps.tile([P, 512], f32, tag="qkvps", name=f"pq{i}")
                       for i in range(2)]
                for s in range(NS):
                    nc.tensor.ldweights(wq_sb[:, s, :, :], perf_mode=DR)
                    for qh in range(2):
                        mm = nc.tensor.matmul(
                            pqs[qh][:, :], wq_sb[:, s, :, :],
                            HT8[2 + qh][:, 2 * s:2 * s + 2, :],
                            start=(s == 0), stop=(s == NS - 1), perf_mode=DR)
                        mm.ldweights = False
                for qh in range(2):
                    # QT = (pq + 64*qb) * (0.125/64)
                    nc.vector.tensor_scalar(
                        QT[pr][:, qh * 512:(qh + 1) * 512], pqs[qh][:, :],
                        qb_sb[:, pr:pr + 1], 0.125 / WS, ALU.add, ALU.mult)
        ht8_es.close()

        for it in range(NT):
            nc.sync.dma_start(out=X[it][:, :],
                              in_=xkv.ap()[1024 + it * P:1024 + (it + 1) * P, :])
        # ---------------- Phase 3: attention ----------------
        o_es = ExitStack()
        o_pool = o_es.enter_context(tc.tile_pool(name="oo", bufs=NT, side="right"))
        O = [o_pool.tile([P, C], bf16, tag="oo", name=f"ot{i}") for i in range(NT)]
        with tc.tile_pool(name="ep", bufs=4) as ep_pool, \
             tc.tile_pool(name="sal", bufs=4) as sal_pool, \
             tc.tile_pool(name="sps", bufs=2, space="PSUM") as sps, \
             tc.tile_pool(name="ops", bufs=4, space="PSUM") as ops:
            for pr in range(NPAIR):
                for ih in range(2):
                    q_lo, q_hi = ih * 512, (ih + 1) * 512
                    # two q-tiles share one opsum bank (single accum group
                    # per bank: one start, one stop)
                    opsum, opcol = {}, {}
                    for g in range(2):
                        t = ops.tile([P, 260], f32, tag="ops", name=f"ops{g}")
                        for k in range(2):
                            it = ih * 4 + g * 2 + k
                            opsum[it] = t
                            opcol[it] = k * 130
                    jmax = ih * 4 + 3 + 8
                    for j in range(jmax + 1):
                        q0 = max((j - 8) * P, q_lo)
                        qlen = q_hi - q0
                        sp = sps.tile([P, 512 + qlen], f32, tag="sps")
                        for hi in range(2):
                            lh = KT[pr][hi * 64:(hi + 1) * 64,
                                         j * P:(j + 1) * P]
                            nc.tensor.matmul(
                                sp[:, hi * 512:hi * 512 + qlen],
                                lh, QT[pr][hi * 64:(hi + 1) * 64, q0:q_hi],
                                start=True, stop=True)
                        ep = ep_pool.tile([P, 2 * qlen], bf16, tag="ep")
                        if qlen == 512:
                            nc.scalar.activation(ep[:, :], sp[:, :], FT.Exp)
                        else:
                            for hi in range(2):
                                nc.scalar.activation(
                                    ep[:, hi * qlen:(hi + 1) * qlen],
                                    sp[:, hi * 512:hi * 512 + qlen], FT.Exp)
                        if j - 8 >= ih * 4 and j >= 8:
                            # diagonal block: mask with transposed-tril
                            for hi in range(2):
                                sl = ep[:, hi * qlen:hi * qlen + P]
                                nc.vector.tensor_mul(sl, sl, tril_sb[:, :])
                        for it in range(max(ih * 4, j - 8), ih * 4 + 4):
                            off = it * P - q0
                            for hi in range(2):
                                # bank group: first writer (even it) starts,
                                # last writer (odd it at its last j) stops
                                nc.tensor.matmul(
                                    opsum[it][:, opcol[it] + hi * 65:
                                              opcol[it] + (hi + 1) * 65],
                                    ep[:, hi * qlen + off:hi * qlen + off + P],
                                    VO[j][:, pr * 130 + hi * 65:
                                          pr * 130 + (hi + 1) * 65],
                                    start=(j == 0 and hi == 0 and (it & 1) == 0),
                                    stop=(j == it + 8 and hi == 1 and
                                          (it & 1) == 1))
                    for it in range(ih * 4, ih * 4 + 4):
                        for hi in range(2):
                            rs = sal_pool.tile([P, 1], f32, tag="rs")
                            # V cols hold 64*V: rs = 1/(64*(rowsum - corr))
                            nc.vector.tensor_scalar(
                                rs[:, :],
                                opsum[it][:, opcol[it] + hi * 65 + 64:
                                          opcol[it] + hi * 65 + 65],
                                corr_sb[:, :], WS, ALU.subtract, ALU.mult)
                            nc.vector.reciprocal(rs[:, :], rs[:, :])
                            nc.vector.tensor_scalar_mul(
                                O[it][:, pr * P + hi * 64:pr * P + hi * 64 + 64],
                                opsum[it][:, opcol[it] + hi * 65:
                                          opcol[it] + hi * 65 + 64], rs[:, :])

        # ---------------- Phase 4: O^T (xbar), Wo fp8, residual ----------------
        with tc.tile_pool(name="ott", bufs=3) as ott_pool, \
             tc.tile_pool(name="ot8", bufs=1) as ot8_pool, \
             tc.tile_pool(name="wos", bufs=1) as wo_pool, \
             tc.tile_pool(name="wops", bufs=2, space="PSUM") as wops:
            OT8 = ot8_pool.tile([P, NKT, 1024], fp8, tag="ot8")
            wo_sb = wo_pool.tile([P, NS, 2, 1024], fp8, tag="wo")
            nc.sync.dma_start(out=wo_sb[:, :, :, :], in_=wo.ap()[:, :])
            for it in range(NT):
                ott = ott_pool.tile([P, NKT, P], bf16, tag="ott")
                nc.scalar.dma_start_transpose(ott[:, :, :], O[it][:, :])
                nc.scalar.copy(out=OT8[:, :, it * P:(it + 1) * P],
                               in_=ott[:, :, :])
            for it in range(NT):
                pw = wops.tile([P, 1024], f32, tag="wops")
                for s in range(NS):
                    nc.tensor.ldweights(
                        OT8[:, 2 * s:2 * s + 2, it * P:(it + 1) * P],
                        perf_mode=DR)
                    for hf in range(2):
                        mm = nc.tensor.matmul(
                            pw[:, hf * 512:(hf + 1) * 512],
                            OT8[:, 2 * s:2 * s + 2, it * P:(it + 1) * P],
                            wo_sb[:, s, :, hf * 512:(hf + 1) * 512],
                            start=(s == 0), stop=(s == NS - 1), perf_mode=DR)
                        mm.ldweights = False
                # X += pw/64 (wo upscaled)
                nc.vector.scalar_tensor_tensor(
                    out=X[it][:, :], in0=pw[:, :], scalar=1.0 / WS,
                    in1=X[it][:, :], op0=ALU.mult, op1=ALU.add)
                if flags["bo"]:
                    nc.vector.tensor_add(X[it][:, :], X[it][:, :], bo_sb[:, :])
        o_es.close()

        # ---------------- Phase 5+6: LN2, FFN (bf16), residual, store ----------
        with tc.tile_pool(name="ln2", bufs=3) as lnp2, \
             tc.tile_pool(name="z2", bufs=3) as zpool2, \
             tc.tile_pool(name="h2t", bufs=2) as h2t_pool, \
             tc.tile_pool(name="ut", bufs=ND) as ut_pool, \
             tc.tile_pool(name="w1s", bufs=8) as w1_pool, \
             tc.tile_pool(name="w2s", bufs=8) as w2_pool, \
             tc.tile_pool(name="ups", bufs=2, space="PSUM") as ups, \
             tc.tile_pool(name="yps", bufs=4, space="PSUM") as yps:
            H2T = [h2t_pool.tile([P, NKT, 512], bf16, tag="h2t",
                                 name=f"h2t{h}") for h in range(2)]
            for it in range(4):
                z2 = ln_tile(X[it], lnp2, zpool2)
                nc.scalar.dma_start_transpose(
                    H2T[0][:, :, it * P:(it + 1) * P], z2[:, :])
            for tch in range(2):
                UT = [ut_pool.tile([P, 512], bf16, tag="ut", name=f"ut{i}") for i in range(ND)]
                for d in range(ND):
                    w1_sb = w1_pool.tile([P, 1024], bf16, tag="w1")
                    nc.sync.dma_start(
                        out=w1_sb[:, :],
                        in_=w1.ap()[:, d * 1024:(d + 1) * 1024])
                    pu = ups.tile([P, 512], f32, tag="ups")
                    for kt in range(NKT):
                        nc.tensor.matmul(
                            pu[:, :], w1_sb[:, kt * P:(kt + 1) * P],
                            H2T[tch][:, kt, :],
                            start=(kt == 0), stop=(kt == NKT - 1))
                    # relu(x + b1) on the ACT engine
                    nc.scalar.activation(UT[d][:, :], pu[:, :], FT.Relu,
                                         bias=b1_sb[:, d:d + 1], scale=1.0)
                if tch == 0:
                    # LN2 for the second half runs on DVE under W1/W2 PE work
                    for it in range(4, 8):
                        z2 = ln_tile(X[it], lnp2, zpool2)
                        nc.scalar.dma_start_transpose(
                            H2T[1][:, :, (it - 4) * P:(it - 3) * P], z2[:, :])
                for ch in range(2):
                    ypsum = [yps.tile([P, 512], f32, tag="yps", name=f"yps{i}")
                             for i in range(4)]
                    for d in range(ND):
                        w2_sb = w2_pool.tile([P, 512], bf16, tag="w2")
                        nc.sync.dma_start(
                            out=w2_sb[:, :],
                            in_=w2.ap()[:, d * 1024 + ch * 512:
                                        d * 1024 + (ch + 1) * 512])
                        for tt in range(4):
                            nc.tensor.matmul(
                                ypsum[tt][:, :],
                                UT[d][:, tt * P:(tt + 1) * P],
                                w2_sb[:, :],
                                start=(d == 0), stop=(d == ND - 1))
                    for tt in range(4):
                        it = tch * 4 + tt
                        xsl = X[it][:, ch * 512:(ch + 1) * 512]
                        nc.vector.tensor_add(xsl, ypsum[tt][:, :], xsl)
                        if flags["b2"]:
                            nc.vector.tensor_add(
                                xsl, xsl, b2_sb[:, ch * 512:(ch + 1) * 512])
                for tt in range(4):
                    it = tch * 4 + tt
                    nc.sync.dma_start(out=out.ap()[it * P:(it + 1) * P, :],
                                      in_=X[it][:, :])

    nc.compile()
    return nc


_CACHE = {}


def _prep(inputs):
    """Host-side preprocessing: fold LN affine into weights, tile/cast, shard."""
    x = np.asarray(inputs["x"], np.float32)
    Wq = np.asarray(inputs["Wq"], np.float32)
    Wk = np.asarray(inputs["Wk"], np.float32)
    Wv = np.asarray(inputs["Wv"], np.float32)
    Wo = np.asarray(inputs["Wo"], np.float32)
    bo = np.asarray(inputs["bo"], np.float32)
    W1 = np.asarray(inputs["W1"], np.float32)
    b1 = np.asarray(inputs["b1"], np.float32)
    W2 = np.asarray(inputs["W2"], np.float32)
    b2 = np.asarray(inputs["b2"], np.float32)
    g1 = np.asarray(inputs["g1"], np.float32)
    be1 = np.asarray(inputs["be1"], np.float32)
    g2 = np.asarray(inputs["g2"], np.float32)
    be2 = np.asarray(inputs["be2"], np.float32)

    Wq_g = (Wq * g1[None, :, None] * WS).astype(E4)   # [16,1024,64], x64
    Wk_g = (Wk * g1[None, :, None] * WS).astype(E4)
    Wv_g = (Wv * g1[None, :, None] * WS).astype(E4)
    Wo_s = (Wo * WS).astype(E4)
    # bias folds use the quantized weights so the compensation is exact
    qb = np.einsum('c,hcd->hd', be1, Wq_g.astype(np.float32))  # 64*qb [16,64]
    kb = np.einsum('c,hcd->hd', be1, Wk_g.astype(np.float32))
    vb = np.einsum('c,hcd->hd', be1, Wv_g.astype(np.float32))
    if np.abs(vb).max() > 0:
        raise NotImplementedError("nonzero folded V bias not supported")

    def lhsT_pack_dr(wflat):
        # [1024 c, 1024 m] -> [128 c_lo, (pair, s, two, 128 m)]
        w = wflat.reshape(NS, 2, P, NPAIR, P)          # [s, two, c_lo, pair, m]
        return np.ascontiguousarray(
            w.transpose(2, 3, 0, 1, 4).reshape(P, 8192))

    def rhs_pack_dr(wflat):
        # [1024 k, 1024 n] -> [128 k_lo, (s, two, 1024 n)]
        w = wflat.reshape(NS, 2, P, 1024)              # [s, two, k_lo, n]
        return np.ascontiguousarray(
            w.transpose(2, 0, 1, 3).reshape(P, 8192))

    wq_h = lhsT_pack_dr(Wq_g.transpose(1, 0, 2).reshape(1024, 1024))
    wk_h = lhsT_pack_dr(Wk_g.transpose(1, 0, 2).reshape(1024, 1024))
    wv_h = rhs_pack_dr(Wv_g.transpose(1, 0, 2).reshape(1024, 1024))
    wo_h = rhs_pack_dr(Wo_s)
    W1_g = (W1 * g2[:, None]).astype(BF16)         # [1024, 4096]
    b1p = b1 + be2 @ W1_g.astype(np.float32)
    w1_h = np.ascontiguousarray(
        W1_g.reshape(8, 128, 32, 128).transpose(1, 2, 0, 3).reshape(128, 32768))
    w2_h = np.ascontiguousarray(
        W2.astype(BF16).reshape(32, 128, 1024).transpose(1, 0, 2).reshape(128, 32768))

    # per-pair stacked [128, 8] bias tables (pre-scaled by 64 via Wq_g/Wk_g)
    qb_t = np.zeros((128, 8), np.float32)
    kb_t = np.zeros((128, 8), np.float32)
    for pr in range(8):
        qb_t[0:64, pr] = qb[2 * pr]
        qb_t[64:128, pr] = qb[2 * pr + 1]
        kb_t[0:64, pr] = kb[2 * pr]
        kb_t[64:128, pr] = kb[2 * pr + 1]
    b1_t = np.ascontiguousarray(b1p.reshape(32, 128).T.astype(np.float32))
    bo_t = np.broadcast_to(bo, (128, 1024)).astype(np.float32).copy()
    b2_t = np.broadcast_to(b2, (128, 1024)).astype(np.float32).copy()

    tril = np.triu(np.ones((128, 128), np.float32)).astype(BF16)

    flags = {"bo": bool(np.abs(bo).max() > 0), "b2": bool(np.abs(b2).max() > 0)}

    shared = dict(wq=wq_h, wk=wk_h, wv=wv_h, wo=wo_h, w1=w1_h, w2=w2_h,
                  trilq=tril, qbias=qb_t, kbias=kb_t,
                  b1p=b1_t, bo_row=bo_t, b2_row=b2_t)
    in_maps = []
    for core in range(8):
        b, half = core // 2, core % 2
        xw = np.zeros((2048, 1024), np.float32)
        if half == 1:
            xw[:1024] = x[b, :1024]
        xw[1024:] = x[b, half * 1024:(half + 1) * 1024]
        cr = np.full((128, 1), 1024.0 if half == 0 else 0.0, np.float32)
        in_maps.append({"xkv": xw, "corr": cr, **shared})
    return in_maps, flags


def _get_nc(flags):
    key = tuple(sorted(flags.items()))
    if key not in _CACHE:
        _CACHE[key] = _build(flags)
    return _CACHE[key]


def run(inputs, **kw):
    in_maps, flags = _prep(inputs)
    nc = _get_nc(flags)
    res = run_bass_kernel_spmd(nc, in_maps, core_ids=list(range(8)), **kw)
    x = np.asarray(inputs["x"], np.float32)
    outf = np.zeros_like(x)
    for core in range(8):
        b, half = core // 2, core % 2
        outf[b, half * 1024:(half + 1) * 1024] = res.results[core]["out"]
    return outf, res


def kernel(**inputs):
    outf, _ = run(inputs)
    return outf
